# revision 33
# baseline (speedup 1.0000x reference)
"""Adaptive embedding (nn_AdaptiveEmbedding) Trainium2 Bass kernel, v2.

Design: one SPMD program with per-core specialized sections dispatched via a
partition_id() If-tree.  Host routes tokens to cores so each core serves a
small set of clusters (cluster-specialized sharding) -- this removes the 8x
replication of the projection matrices that dominated HBM traffic in v1.

Per 128-token block, the device:
  Pool : one indirect_dma_start (HW dynamic-offset DMA, no ucode library)
         gathering 128 table rows -> SBUF [tokens, d] (token per partition)
  PE   : transposes the gathered tile chunk-wise to [d, tokens] (via identity
         matmul into bf16 PSUM), then runs the projection matmuls into f32
         PSUM, software-pipelined one block ahead of the transposes
  DVE  : copies transposed chunks PSUM->SBUF and casts output half 0
  ACT  : casts output half 1
  SP   : loads idx/identity/projection tiles, stores finished blocks

Host scatters per-core block outputs back to original token positions.
"""

import math
from contextlib import ExitStack

import numpy as np
import ml_dtypes

import concourse.bacc as bacc
import concourse.bass as bass
import concourse.mybir as mybir
from concourse.bass_utils import run_bass_kernel_spmd

N_CORES = 8
D_PROJ = 1024
EMB_SCALE = float(D_PROJ) ** 0.5
BF16 = ml_dtypes.bfloat16

# clusters: (token_left, token_right, d)
CLUSTERS = [
    (0, 20000, 1024),
    (20000, 60000, 256),
    (60000, 100000, 64),
    (100000, 128000, 16),
]

# cost model for the balancer (ns)
POOL_PER_BLOCK = 1410.0
PE_NS_PER_COL = 0.55         # p-state mix
DMA_NS_PER_BYTE = 1.0 / 300.0e9 * 1e9   # 300 GB/s
POOL_START, PE_START, DMA_START = 10500.0, 14500.0, 8300.0
VEC_START = 14500.0
# max number of cores each cluster's blocks (and proj copy) may spread to
SPREAD_CAP = [4, 3, 4, 6]


def _dve_ns(d):
    return max(d // 128, 1) * 300.0 + 530.0  # T-copies + h0 cast

NB_MAX = 16          # max blocks per core the program supports
G_COLS = 8192        # gather buffer cols (bf16) per partition
NPS = 2              # psum double-buffer depth (out tiles and T tiles)
NO_DISPATCH = False  # debug: emit plans[0] for every core, no branching

LAST_RESULT = None


def _pe_cols(d):
    nch = max(d // 128, 1)
    kd = min(d, 128)
    return nch * 128 + nch * 2 * 512  # transposes + matmuls (free-dim cols)


def _block_bytes(d):
    return 128 * d * 2 + 128 * D_PROJ * 2  # gather + out (bf16)


PJ_BYTES = [2 * 1024 * 1024, 512 * 1024, 128 * 1024, 32 * 1024]


def _route(flat):
    """Token routing per cluster: sorted positions and local indices."""
    out = []
    for (l, r, d) in CLUSTERS:
        sel = (flat >= l) & (flat < r)
        pos = np.nonzero(sel)[0]
        loc = (flat[pos] - l).astype(np.int64)
        order = np.argsort(loc, kind="stable")
        out.append({"pos": pos[order], "loc": loc[order], "d": d, "n": len(pos)})
    return out


def _balance(rt):
    """Block assignment minimizing max per-core makespan, with a hard cap on
    how many cores each cluster (and its proj copy) may spread to.

    Returns cores: list of 8 dicts with
      blocks: list of (cluster, start, end) token ranges (<=128 each)
      pj: set of cluster ids present
    Token ranges index into the cluster's sorted token arrays, so each
    core gets a contiguous slice of the sorted-by-loc token list (compact
    vocab slice per core).
    """
    cores = [{"blocks": [], "pj": set(), "pool": 0.0, "pe": 0.0, "dma": 0.0,
              "dve": 0.0, "nb": 0}
             for _ in range(N_CORES)]

    def span(c, dpool=0.0, dpe=0.0, ddma=0.0, pj_extra=0, ddve=0.0):
        pjb = sum(PJ_BYTES[p] for p in c["pj"]) + pj_extra
        return max(POOL_START + c["pool"] + dpool,
                   PE_START + c["pe"] + dpe,
                   VEC_START + c["dve"] + ddve,
                   DMA_START + (c["dma"] + ddma + pjb) * DMA_NS_PER_BYTE)

    for ci in [0, 1, 2, 3]:
        n = rt[ci]["n"]
        nblk = (n + 127) // 128
        d = rt[ci]["d"]
        pe_b = _pe_cols(d) * PE_NS_PER_COL
        by_b = _block_bytes(d)
        dve_b = _dve_ns(d)
        counts = [0] * N_CORES
        for _ in range(nblk):
            # candidate cores: those already serving ci, or (if spread cap
            # not hit) any core.  Cores serving c0 are PE/DMA-heavy: cap
            # their total block count so the gather cadence of small blocks
            # does not stack on top of the c0 matmul load.
            have = [k for k in range(N_CORES) if ci in cores[k]["pj"]]
            cands = list(range(N_CORES)) if len(have) < SPREAD_CAP[ci] else have
            if ci != 0:
                cands = [k for k in cands
                         if not (0 in cores[k]["pj"]
                                 and cores[k]["nb"] >= 6)] or cands
            best, bestv = None, None
            for k in cands:
                c = cores[k]
                extra_pj = 0 if ci in c["pj"] else PJ_BYTES[ci]
                v = span(c, POOL_PER_BLOCK, pe_b, by_b, extra_pj, dve_b)
                # tie-break: prefer cores that already carry this proj
                v += (0 if ci in c["pj"] else 1.0)
                if bestv is None or v < bestv - 1e-9:
                    bestv, best = v, k
            c = cores[best]
            c["pool"] += POOL_PER_BLOCK
            c["pe"] += pe_b
            c["dma"] += by_b
            c["dve"] += dve_b
            c["pj"].add(ci)
            c["nb"] += 1
            counts[best] += 1
        start = 0
        for k in range(N_CORES):
            for _ in range(counts[k]):
                end = min(start + 128, n)
                cores[k]["blocks"].append((ci, start, end))
                start = end
        assert start == n
    return cores


def _build(plans):
    """plans[k]: list of block descriptors:
       (cluster, d, tab_row_offset_base, nblk_index, mm) plus idx data handled
       by host.  We need per-core: blocks list with (cluster, mm)."""
    nc = bacc.Bacc("TRN2", target_bir_lowering=False, num_devices=N_CORES)

    # table shapes: max rows over cores per cluster (host pads)
    tab_rows = [max((p["tab_rows"][ci] for p in plans), default=1) or 1
                for ci in range(4)]
    tabs = [nc.dram_tensor(f"tab{ci}", [max(tab_rows[ci], 1), CLUSTERS[ci][2]],
                           mybir.dt.bfloat16, kind="ExternalInput")
            for ci in range(4)]
    idxd = nc.dram_tensor("idxd", [128, NB_MAX], mybir.dt.int32,
                          kind="ExternalInput")
    iden = nc.dram_tensor("iden", [128, 128], mybir.dt.bfloat16,
                          kind="ExternalInput")
    pjts = [nc.dram_tensor(f"pjt{ci}", [min(CLUSTERS[ci][2], 128),
                                        max(CLUSTERS[ci][2] // 128, 1) * D_PROJ],
                           mybir.dt.bfloat16, kind="ExternalInput")
            for ci in range(4)]
    outD = nc.dram_tensor("out", [NB_MAX * 128, D_PROJ], mybir.dt.bfloat16,
                          kind="ExternalOutput")

    stack = ExitStack()
    sb = lambda name, shape, dt: stack.enter_context(
        nc.sbuf_tensor(name, list(shape), dt))
    pt_ = lambda name, shape, dt: stack.enter_context(
        nc.psum_tensor(name, list(shape), dt))
    sem = lambda name: stack.enter_context(nc.semaphore(name))

    with stack:
        idxt = sb("idxt", [128, NB_MAX], mybir.dt.int32)
        idn = sb("idn", [128, 128], mybir.dt.bfloat16)
        G = sb("G", [128, G_COLS], mybir.dt.bfloat16)
        ET = [sb(f"ET{i}", [128, 1024], mybir.dt.bfloat16) for i in range(NPS)]
        OG = sb("OG", [128, NB_MAX * D_PROJ], mybir.dt.bfloat16)
        pjt_sb = [sb(f"pj{ci}", [min(CLUSTERS[ci][2], 128),
                                 max(CLUSTERS[ci][2] // 128, 1) * D_PROJ],
                     mybir.dt.bfloat16) for ci in range(4)]
        # transposes must land at a PSUM bank base: rotate four bank-sized
        # slots, each transpose writes cols 0:128 of its slot
        NPT = 4
        psT = [pt_(f"psT{i}", [128, 1024], mybir.dt.bfloat16)
               for i in range(NPT)]
        psO = [pt_(f"psO{i}", [128, D_PROJ], mybir.dt.float32)
               for i in range(NPS)]

        isem = sem("isem")    # idx load
        idsem = sem("idsem")  # identity load
        psem = [sem(f"psem{i}") for i in range(7)]   # proj tile loads
        gsem = [sem(f"gsem{i}") for i in range(NB_MAX)]  # per-block gathers
        tsem = sem("tsem")    # PE transposes
        csem = sem("csem")    # DVE chunk copies
        mmsem = sem("mmsem")  # matmul halves
        vcsem = sem("vcsem")  # DVE out casts (h0)
        acsem = sem("acsem")  # ACT out casts (h1)
        osem = sem("osem")    # stores

        # per-core proj DMA schedule: list of (cluster, chunk_lo, n_chunks)
        # c0 is split into 4 DMAs of 2 chunks; others one DMA each.
        def proj_dmas(pjset):
            sched = []
            for ci in sorted(pjset):
                nch = max(CLUSTERS[ci][2] // 128, 1)
                if ci == 0:
                    for c0 in range(0, nch, 2):
                        sched.append((ci, c0, 2))
                else:
                    sched.append((ci, 0, nch))
            return sched

        def section_sync(sy, plan):
            for i, (ci, c0, w) in enumerate(proj_dmas(plan["pj"])):
                part = min(CLUSTERS[ci][2], 128)
                sy.dma_start(
                    pjt_sb[ci][:part, c0 * D_PROJ:(c0 + w) * D_PROJ],
                    pjts[ci][:part, c0 * D_PROJ:(c0 + w) * D_PROJ],
                ).then_inc(psem[i], 16)
            # stores: full-width for all but the last block; the last block
            # ships each half as soon as its cast lands (shorter tail)
            nb = len(plan["blocks"])
            nst = 0
            for b, blk in enumerate(plan["blocks"]):
                if b < nb - 1:
                    sy.wait_ge(vcsem, b + 1)
                    sy.wait_ge(acsem, b + 1)
                    sy.dma_start(
                        outD[b * 128: b * 128 + blk["mm"], :],
                        OG[:blk["mm"], b * D_PROJ:(b + 1) * D_PROJ],
                    ).then_inc(osem, 16)
                    nst += 1
                else:
                    sy.wait_ge(vcsem, b + 1)
                    sy.dma_start(
                        outD[b * 128: b * 128 + blk["mm"], 0:512],
                        OG[:blk["mm"], b * D_PROJ: b * D_PROJ + 512],
                    ).then_inc(osem, 16)
                    sy.wait_ge(acsem, b + 1)
                    sy.dma_start(
                        outD[b * 128: b * 128 + blk["mm"], 512:1024],
                        OG[:blk["mm"], b * D_PROJ + 512:(b + 1) * D_PROJ],
                    ).then_inc(osem, 16)
                    nst += 2
            sy.wait_ge(osem, 16 * nst)

        def section_pool(gp, plan):
            gp.wait_ge(isem, 16)
            for b, blk in enumerate(plan["blocks"]):
                ci = blk["ci"]
                d = CLUSTERS[ci][2]
                gp.indirect_dma_start(
                    G[:, blk["goff"]: blk["goff"] + d], None,
                    tabs[ci][:, :],
                    bass.IndirectOffsetOnAxis(ap=idxt[:, b:b + 1], axis=0),
                ).then_inc(gsem[b], 16)

        def section_pe(te, plan):
            blocks = plan["blocks"]
            pj_sched = proj_dmas(plan["pj"])
            # dma index (0-based) needed for cluster ci chunk c
            def pj_need(ci, c):
                for i, (cj, c0, w) in enumerate(pj_sched):
                    if cj == ci and c0 <= c < c0 + w:
                        return i
                raise AssertionError

            te.wait_ge(idsem, 16)

            cum_copies = [0] * (len(blocks) + 1)
            for b, blk in enumerate(blocks):
                d = CLUSTERS[blk["ci"]][2]
                cum_copies[b + 1] = cum_copies[b] + max(d // 128, 1)

            def emit_T_chunks(b, c_lo, c_hi):
                blk = blocks[b]
                d = CLUSTERS[blk["ci"]][2]
                if c_lo == 0:
                    te.wait_ge(gsem[b], 16)
                for c in range(c_lo, c_hi):
                    w = min(128, d - c * 128)
                    t = cum_copies[b] + c
                    if t >= NPT:
                        te.wait_ge(csem, t - (NPT - 1))
                    te.transpose(
                        psT[t % NPT][:w, 0:128],
                        G[:, blk["goff"] + c * 128: blk["goff"] + c * 128 + w],
                        idn[:, :],
                    ).then_inc(tsem, 1)

            def emit_MM_half(b, h):
                """One accumulation group (half h of block b)."""
                blk = blocks[b]
                ci = blk["ci"]
                d = CLUSTERS[ci][2]
                nch = max(d // 128, 1)
                kd = min(d, 128)
                mm = blk["mm"]
                if h == 0 and b >= NPS:
                    # psO reuse: casts of block b-NPS must be done
                    te.wait_ge(vcsem, b - NPS + 1)
                    te.wait_ge(acsem, b - NPS + 1)
                seen = plan.setdefault("_pj_seen", set())
                last = None
                for c in range(nch):
                    if h == 0:
                        te.wait_ge(csem, cum_copies[b] + c + 1)
                    i_pj = pj_need(ci, c)
                    if (ci, i_pj) not in seen:
                        te.wait_ge(psem[i_pj], 16)
                        seen.add((ci, i_pj))
                    last = te.matmul(
                        psO[b % NPS][:mm, h * 512:(h + 1) * 512],
                        ET[b % NPS][:kd, c * 128: c * 128 + mm],
                        pjt_sb[ci][:kd, c * D_PROJ + h * 512:
                                   c * D_PROJ + h * 512 + 512],
                        start=(c == 0),
                        stop=(c == nch - 1),
                    )
                last.then_inc(mmsem, 1)

            # software pipeline: T(0) up front; then per block b the two
            # matmul groups with the NEXT block's transposes emitted at the
            # group boundaries (PSUM groups never interleave).
            nb = len(blocks)
            emit_T_chunks(0, 0, cum_copies[1] - cum_copies[0])
            for b in range(nb):
                nch_next = (cum_copies[b + 2] - cum_copies[b + 1]
                            if b + 1 < nb else 0)
                emit_MM_half(b, 0)
                if nch_next:
                    emit_T_chunks(b + 1, 0, (nch_next + 1) // 2)
                emit_MM_half(b, 1)
                if nch_next:
                    emit_T_chunks(b + 1, (nch_next + 1) // 2, nch_next)

        def section_dve(ve, plan):
            blocks = plan["blocks"]
            NPT = 4
            nt = 0

            def copy_chunks(b):
                nonlocal nt
                blk = blocks[b]
                d = CLUSTERS[blk["ci"]][2]
                nch = max(d // 128, 1)
                for c in range(nch):
                    w = min(128, d - c * 128)
                    ve.wait_ge(tsem, nt + 1)
                    ve.tensor_copy(
                        ET[b % NPS][:w, c * 128:(c + 1) * 128],
                        psT[nt % NPT][:w, 0:128],
                    ).then_inc(csem, 1)
                    nt += 1

            # copies first, then the cast of the previous block: keeps the
            # per-block PE<->DVE chain shorter than the gather cadence
            copy_chunks(0)
            for b, blk in enumerate(blocks):
                mm = blk["mm"]
                if b + 1 < len(blocks):
                    copy_chunks(b + 1)
                ve.wait_ge(mmsem, 2 * b + 1)
                ve.tensor_copy(
                    OG[:mm, b * D_PROJ: b * D_PROJ + 512],
                    psO[b % NPS][:mm, 0:512],
                ).then_inc(vcsem, 1)

        def section_act(sc, plan):
            blocks = plan["blocks"]
            for b, blk in enumerate(blocks):
                mm = blk["mm"]
                sc.wait_ge(mmsem, 2 * (b + 1))
                sc.copy(
                    OG[:mm, b * D_PROJ + 512: b * D_PROJ + 1024],
                    psO[b % NPS][:mm, 512:1024],
                ).then_inc(acsem, 1)

        def dispatch(eng, emit):
            if NO_DISPATCH:
                emit(eng, plans[0])
                return
            pid = eng.partition_id()
            with eng.If_lt(pid, 4):
                with eng.If_lt(pid, 2):
                    with eng.If_eq(pid, 0):
                        emit(eng, plans[0])
                    with eng.Else():
                        emit(eng, plans[1])
                with eng.Else():
                    with eng.If_eq(pid, 2):
                        emit(eng, plans[2])
                    with eng.Else():
                        emit(eng, plans[3])
            with eng.Else():
                with eng.If_lt(pid, 6):
                    with eng.If_eq(pid, 4):
                        emit(eng, plans[4])
                    with eng.Else():
                        emit(eng, plans[5])
                with eng.Else():
                    with eng.If_eq(pid, 6):
                        emit(eng, plans[6])
                    with eng.Else():
                        emit(eng, plans[7])

        def _(sy):
            # idx + identity loads are identical on every core: issue them
            # before the dispatch tree so they are not delayed by the
            # partition-id load
            sy.dma_start(idxt[:, :], idxd[:, :]).then_inc(isem, 16)
            sy.dma_start(idn[:, :], iden[:, :]).then_inc(idsem, 16)
            dispatch(sy, section_sync)
        _(nc.sync)

        def _(gp):
            dispatch(gp, section_pool)
        _(nc.gpsimd)

        def _(te):
            def emit(eng, plan):
                plan.pop("_pj_seen", None)
                section_pe(eng, plan)
            dispatch(te, emit)
        _(nc.tensor)

        def _(ve):
            dispatch(ve, section_dve)
        _(nc.vector)

        def _(sc):
            dispatch(sc, section_act)
        _(nc.scalar)

        nc.compile()
    return nc


def kernel(input, emb0, emb1, emb2, emb3, proj0, proj1, proj2, proj3):
    global LAST_RESULT
    inp = np.asarray(input)
    flat = inp.reshape(-1).astype(np.int64)
    T = flat.shape[0]
    tables = [np.asarray(emb0), np.asarray(emb1), np.asarray(emb2),
              np.asarray(emb3)]
    projs = [np.asarray(proj0), np.asarray(proj1), np.asarray(proj2),
             np.asarray(proj3)]

    rt = _route(flat)
    cores = _balance(rt)

    # Cores 3 and 7 showed a consistent extra store-drain latency on HW:
    # hand them the two lightest plans.
    def plan_cost(c):
        return max(POOL_START + c["pool"], PE_START + c["pe"],
                   DMA_START + (c["dma"] + sum(PJ_BYTES[p] for p in c["pj"]))
                   * DMA_NS_PER_BYTE)

    order = sorted(range(N_CORES), key=lambda k: -plan_cost(cores[k]))
    pid_pref = [1, 2, 4, 5, 6, 0, 3, 7]  # heaviest five -> 1,2,4,5,6
    perm = [None] * N_CORES
    for rank, k in enumerate(order):
        perm[pid_pref[rank]] = k
    cores = [cores[perm[pid]] for pid in range(N_CORES)]

    # --- build per-core plans ---------------------------------------------
    plans = []
    for k in range(N_CORES):
        c = cores[k]
        blocks = []
        goff = 0
        tab_lo = {}  # cluster -> (lo_loc, hi_loc)
        for (ci, s, e) in c["blocks"]:
            loc = rt[ci]["loc"][s:e]
            lo, hi = tab_lo.get(ci, (1 << 60, -1))
            tab_lo[ci] = (min(lo, int(loc.min())), max(hi, int(loc.max())))
        plan = {"pj": c["pj"], "blocks": [], "tab_rows": [0] * 4,
                "tab_base": {}}
        for ci, (lo, hi) in tab_lo.items():
            plan["tab_base"][ci] = lo
            plan["tab_rows"][ci] = hi - lo + 1
        for (ci, s, e) in c["blocks"]:
            d = CLUSTERS[ci][2]
            plan["blocks"].append({
                "ci": ci, "s": s, "e": e, "mm": e - s, "goff": goff,
            })
            goff += d
        assert goff <= G_COLS, f"core {k}: G overflow {goff}"
        assert len(plan["blocks"]) <= NB_MAX
        plans.append(plan)

    nc = _build(plans)

    # --- stage host data ---------------------------------------------------
    tab_rows_max = [max(max((p["tab_rows"][ci] for p in plans)), 1)
                    for ci in range(4)]
    pjt_stage = []
    for ci in range(4):
        d = CLUSTERS[ci][2]
        pt = projs[ci].T.astype(np.float32) * EMB_SCALE  # [d, D_PROJ]
        if d >= 128:
            nch = d // 128
            pt = pt.reshape(nch, 128, D_PROJ).transpose(1, 0, 2)
            pt = pt.reshape(128, nch * D_PROJ)
        pjt_stage.append(np.ascontiguousarray(pt.astype(BF16)))
    iden_np = np.eye(128, dtype=np.float32).astype(BF16)

    in_maps = []
    for k in range(N_CORES):
        plan = plans[k]
        mm = {"iden": iden_np}
        for ci in range(4):
            rows = tab_rows_max[ci]
            d = CLUSTERS[ci][2]
            arr = np.zeros((rows, d), dtype=BF16)
            if plan["tab_rows"][ci] > 0:
                base = plan["tab_base"][ci]
                n = plan["tab_rows"][ci]
                arr[:n] = tables[ci][base: base + n].astype(BF16)
            mm[f"tab{ci}"] = arr
            mm[f"pjt{ci}"] = pjt_stage[ci]
        idx = np.zeros((128, NB_MAX), dtype=np.int32)
        for b, blk in enumerate(plan["blocks"]):
            ci = blk["ci"]
            loc = rt[ci]["loc"][blk["s"]: blk["e"]] - plan["tab_base"][ci]
            idx[: blk["mm"], b] = loc.astype(np.int32)
        mm["idxd"] = idx
        in_maps.append(mm)

    res = run_bass_kernel_spmd(nc, in_maps, core_ids=list(range(N_CORES)))
    LAST_RESULT = res

    # --- unpermute ---------------------------------------------------------
    out_full = np.zeros((T, D_PROJ), np.float32)
    for k in range(N_CORES):
        rows = res.results[k]["out"]
        for b, blk in enumerate(plans[k]["blocks"]):
            ci = blk["ci"]
            pos = rt[ci]["pos"][blk["s"]: blk["e"]]
            out_full[pos] = rows[b * 128: b * 128 + blk["mm"]].astype(
                np.float32)
    return out_full.reshape(*inp.shape, D_PROJ)


# revision 36
# speedup vs baseline: 1.1312x; 1.1312x over previous
"""Adaptive embedding (nn_AdaptiveEmbedding) Trainium2 Bass kernel, v2.

Design: one SPMD program with per-core specialized sections dispatched via a
partition_id() If-tree.  Host routes tokens to cores so each core serves a
small set of clusters (cluster-specialized sharding) -- this removes the 8x
replication of the projection matrices that dominated HBM traffic in v1.

Per 128-token block, the device:
  Pool : one indirect_dma_start (HW dynamic-offset DMA, no ucode library)
         gathering 128 table rows -> SBUF [tokens, d] (token per partition)
  PE   : transposes the gathered tile chunk-wise to [d, tokens] (via identity
         matmul into bf16 PSUM), then runs the projection matmuls into f32
         PSUM, software-pipelined one block ahead of the transposes
  DVE  : copies transposed chunks PSUM->SBUF and casts output half 0
  ACT  : casts output half 1
  SP   : loads idx/identity/projection tiles, stores finished blocks

Host scatters per-core block outputs back to original token positions.
"""

import math
from contextlib import ExitStack

import numpy as np
import ml_dtypes

import concourse.bacc as bacc
import concourse.bass as bass
import concourse.mybir as mybir
from concourse.bass_utils import run_bass_kernel_spmd

N_CORES = 8
D_PROJ = 1024
EMB_SCALE = float(D_PROJ) ** 0.5
BF16 = ml_dtypes.bfloat16

# clusters: (token_left, token_right, d)
CLUSTERS = [
    (0, 20000, 1024),
    (20000, 60000, 256),
    (60000, 100000, 64),
    (100000, 128000, 16),
]

# cost model for the balancer (ns)
POOL_PER_BLOCK = 1410.0
PE_NS_PER_COL = 0.55         # p-state mix
DMA_NS_PER_BYTE = 1.0 / 300.0e9 * 1e9   # 300 GB/s
POOL_START, PE_START, DMA_START = 10500.0, 14500.0, 8300.0
VEC_START = 14500.0
# max number of cores each cluster's blocks (and proj copy) may spread to
SPREAD_CAP = [4, 3, 4, 6]


def _dve_ns(d):
    return max(d // 128, 1) * 300.0 + 530.0  # T-copies + h0 cast

NB_MAX = 16          # max blocks per core the program supports
G_COLS = 8192        # gather buffer cols (bf16) per partition
NPS = 2              # psum double-buffer depth (out tiles and T tiles)
NO_DISPATCH = False  # debug: emit plans[0] for every core, no branching

LAST_RESULT = None


def _pe_cols(d):
    nch = max(d // 128, 1)
    kd = min(d, 128)
    return nch * 128 + nch * 2 * 512  # transposes + matmuls (free-dim cols)


def _block_bytes(d):
    return 128 * d * 2 + 128 * D_PROJ * 2  # gather + out (bf16)


PJ_BYTES = [2 * 1024 * 1024, 512 * 1024, 128 * 1024, 32 * 1024]


def _route(flat):
    """Token routing per cluster: sorted positions and local indices."""
    out = []
    for (l, r, d) in CLUSTERS:
        sel = (flat >= l) & (flat < r)
        pos = np.nonzero(sel)[0]
        loc = (flat[pos] - l).astype(np.int64)
        order = np.argsort(loc, kind="stable")
        out.append({"pos": pos[order], "loc": loc[order], "d": d, "n": len(pos)})
    return out


def _balance(rt):
    """Block assignment minimizing max per-core makespan, with a hard cap on
    how many cores each cluster (and its proj copy) may spread to.

    Returns cores: list of 8 dicts with
      blocks: list of (cluster, start, end) token ranges (<=128 each)
      pj: set of cluster ids present
    Token ranges index into the cluster's sorted token arrays, so each
    core gets a contiguous slice of the sorted-by-loc token list (compact
    vocab slice per core).
    """
    cores = [{"blocks": [], "pj": set(), "pool": 0.0, "pe": 0.0, "dma": 0.0,
              "dve": 0.0, "nb": 0}
             for _ in range(N_CORES)]

    def span(c, dpool=0.0, dpe=0.0, ddma=0.0, pj_extra=0, ddve=0.0):
        pjb = sum(PJ_BYTES[p] for p in c["pj"]) + pj_extra
        return max(POOL_START + c["pool"] + dpool,
                   PE_START + c["pe"] + dpe,
                   VEC_START + c["dve"] + ddve,
                   DMA_START + (c["dma"] + ddma + pjb) * DMA_NS_PER_BYTE)

    for ci in [0, 1, 2, 3]:
        n = rt[ci]["n"]
        nblk = (n + 127) // 128
        d = rt[ci]["d"]
        pe_b = _pe_cols(d) * PE_NS_PER_COL
        by_b = _block_bytes(d)
        dve_b = _dve_ns(d)
        # spread floor so per-core G columns and block counts stay in range
        blocks_cap = max(min(G_COLS // d, NB_MAX) - 2, 1)
        spread = max(SPREAD_CAP[ci], -(-nblk // blocks_cap))
        counts = [0] * N_CORES
        for _ in range(nblk):
            # candidate cores: those already serving ci, or (if spread cap
            # not hit) any core.  Cores serving c0 are PE/DMA-heavy: cap
            # their total block count so the gather cadence of small blocks
            # does not stack on top of the c0 matmul load.
            have = [k for k in range(N_CORES)
                    if ci in cores[k]["pj"]
                    and counts[k] < blocks_cap
                    and cores[k]["nb"] < NB_MAX - 1]
            cands = (list(range(N_CORES)) if len(have) < spread else have)
            cands = [k for k in cands
                     if counts[k] < blocks_cap
                     and cores[k]["nb"] < NB_MAX - 1] or \
                    [k for k in range(N_CORES) if cores[k]["nb"] < NB_MAX - 1]
            if ci != 0:
                cands = [k for k in cands
                         if not (0 in cores[k]["pj"]
                                 and cores[k]["nb"] >= 6)] or cands
            best, bestv = None, None
            for k in cands:
                c = cores[k]
                extra_pj = 0 if ci in c["pj"] else PJ_BYTES[ci]
                v = span(c, POOL_PER_BLOCK, pe_b, by_b, extra_pj, dve_b)
                # tie-break: prefer cores that already carry this proj
                v += (0 if ci in c["pj"] else 1.0)
                if bestv is None or v < bestv - 1e-9:
                    bestv, best = v, k
            c = cores[best]
            c["pool"] += POOL_PER_BLOCK
            c["pe"] += pe_b
            c["dma"] += by_b
            c["dve"] += dve_b
            c["pj"].add(ci)
            c["nb"] += 1
            counts[best] += 1
        start = 0
        for k in range(N_CORES):
            for _ in range(counts[k]):
                end = min(start + 128, n)
                cores[k]["blocks"].append((ci, start, end))
                start = end
        assert start == n
    return cores


def _build(plans):
    """plans[k]: list of block descriptors:
       (cluster, d, tab_row_offset_base, nblk_index, mm) plus idx data handled
       by host.  We need per-core: blocks list with (cluster, mm)."""
    nc = bacc.Bacc("TRN2", target_bir_lowering=False, num_devices=N_CORES)

    # table shapes: max rows over cores per cluster (host pads)
    tab_rows = [max((p["tab_rows"][ci] for p in plans), default=1) or 1
                for ci in range(4)]
    tabs = [nc.dram_tensor(f"tab{ci}", [max(tab_rows[ci], 1), CLUSTERS[ci][2]],
                           mybir.dt.bfloat16, kind="ExternalInput")
            for ci in range(4)]
    idxd = nc.dram_tensor("idxd", [128, NB_MAX], mybir.dt.int32,
                          kind="ExternalInput")
    iden = nc.dram_tensor("iden", [128, 128], mybir.dt.bfloat16,
                          kind="ExternalInput")
    pjts = [nc.dram_tensor(f"pjt{ci}", [min(CLUSTERS[ci][2], 128),
                                        max(CLUSTERS[ci][2] // 128, 1) * D_PROJ],
                           mybir.dt.bfloat16, kind="ExternalInput")
            for ci in range(4)]
    outD = nc.dram_tensor("out", [NB_MAX * 128, D_PROJ], mybir.dt.bfloat16,
                          kind="ExternalOutput")

    stack = ExitStack()
    sb = lambda name, shape, dt: stack.enter_context(
        nc.sbuf_tensor(name, list(shape), dt))
    pt_ = lambda name, shape, dt: stack.enter_context(
        nc.psum_tensor(name, list(shape), dt))
    sem = lambda name: stack.enter_context(nc.semaphore(name))

    with stack:
        idxt = sb("idxt", [128, NB_MAX], mybir.dt.int32)
        idn = sb("idn", [128, 128], mybir.dt.bfloat16)
        G = sb("G", [128, G_COLS], mybir.dt.bfloat16)
        ET = [sb(f"ET{i}", [128, 1024], mybir.dt.bfloat16) for i in range(NPS)]
        OG = sb("OG", [128, NB_MAX * D_PROJ], mybir.dt.bfloat16)
        pjt_sb = [sb(f"pj{ci}", [min(CLUSTERS[ci][2], 128),
                                 max(CLUSTERS[ci][2] // 128, 1) * D_PROJ],
                     mybir.dt.bfloat16) for ci in range(4)]
        # transposes must land at a PSUM bank base: rotate four bank-sized
        # slots, each transpose writes cols 0:128 of its slot
        NPT = 4
        psT = [pt_(f"psT{i}", [128, 1024], mybir.dt.bfloat16)
               for i in range(NPT)]
        psO = [pt_(f"psO{i}", [128, D_PROJ], mybir.dt.float32)
               for i in range(NPS)]

        isem = sem("isem")    # idx load
        idsem = sem("idsem")  # identity load
        psem = [sem(f"psem{i}") for i in range(7)]   # proj tile loads
        gsem = [sem(f"gsem{i}") for i in range(NB_MAX)]  # per-block gathers
        tsem = sem("tsem")    # PE transposes
        csem = sem("csem")    # DVE chunk copies
        mmsem = sem("mmsem")  # matmul halves
        vcsem = sem("vcsem")  # DVE out casts (h0)
        acsem = sem("acsem")  # ACT out casts (h1)
        osem = sem("osem")    # stores

        # per-core proj DMA schedule: list of (cluster, chunk_lo, n_chunks)
        # c0 is split into 4 DMAs of 2 chunks; others one DMA each.
        def proj_dmas(pjset):
            sched = []
            for ci in sorted(pjset):
                nch = max(CLUSTERS[ci][2] // 128, 1)
                if ci == 0:
                    for c0 in range(0, nch, 2):
                        sched.append((ci, c0, 2))
                else:
                    sched.append((ci, 0, nch))
            return sched

        def section_sync(sy, plan):
            for i, (ci, c0, w) in enumerate(proj_dmas(plan["pj"])):
                part = min(CLUSTERS[ci][2], 128)
                sy.dma_start(
                    pjt_sb[ci][:part, c0 * D_PROJ:(c0 + w) * D_PROJ],
                    pjts[ci][:part, c0 * D_PROJ:(c0 + w) * D_PROJ],
                ).then_inc(psem[i], 16)
            # stores: full-width for all but the last block; the last block
            # ships each half as soon as its cast lands (shorter tail)
            nb = len(plan["blocks"])
            nst = 0
            for b, blk in enumerate(plan["blocks"]):
                if b < nb - 1:
                    sy.wait_ge(vcsem, b + 1)
                    sy.wait_ge(acsem, b + 1)
                    sy.dma_start(
                        outD[b * 128: b * 128 + blk["mm"], :],
                        OG[:blk["mm"], b * D_PROJ:(b + 1) * D_PROJ],
                    ).then_inc(osem, 16)
                    nst += 1
                else:
                    sy.wait_ge(vcsem, b + 1)
                    sy.dma_start(
                        outD[b * 128: b * 128 + blk["mm"], 0:512],
                        OG[:blk["mm"], b * D_PROJ: b * D_PROJ + 512],
                    ).then_inc(osem, 16)
                    sy.wait_ge(acsem, b + 1)
                    sy.dma_start(
                        outD[b * 128: b * 128 + blk["mm"], 512:1024],
                        OG[:blk["mm"], b * D_PROJ + 512:(b + 1) * D_PROJ],
                    ).then_inc(osem, 16)
                    nst += 2
            sy.wait_ge(osem, 16 * nst)

        def section_pool(gp, plan):
            gp.wait_ge(isem, 16)
            for b, blk in enumerate(plan["blocks"]):
                ci = blk["ci"]
                d = CLUSTERS[ci][2]
                gp.indirect_dma_start(
                    G[:, blk["goff"]: blk["goff"] + d], None,
                    tabs[ci][:, :],
                    bass.IndirectOffsetOnAxis(ap=idxt[:, b:b + 1], axis=0),
                ).then_inc(gsem[b], 16)

        def section_pe(te, plan):
            blocks = plan["blocks"]
            pj_sched = proj_dmas(plan["pj"])
            # dma index (0-based) needed for cluster ci chunk c
            def pj_need(ci, c):
                for i, (cj, c0, w) in enumerate(pj_sched):
                    if cj == ci and c0 <= c < c0 + w:
                        return i
                raise AssertionError

            te.wait_ge(idsem, 16)

            cum_copies = [0] * (len(blocks) + 1)
            for b, blk in enumerate(blocks):
                d = CLUSTERS[blk["ci"]][2]
                cum_copies[b + 1] = cum_copies[b] + max(d // 128, 1)

            def emit_T_chunks(b, c_lo, c_hi):
                blk = blocks[b]
                d = CLUSTERS[blk["ci"]][2]
                if c_lo == 0:
                    te.wait_ge(gsem[b], 16)
                for c in range(c_lo, c_hi):
                    w = min(128, d - c * 128)
                    t = cum_copies[b] + c
                    if t >= NPT:
                        te.wait_ge(csem, t - (NPT - 1))
                    te.transpose(
                        psT[t % NPT][:w, 0:128],
                        G[:, blk["goff"] + c * 128: blk["goff"] + c * 128 + w],
                        idn[:, :],
                    ).then_inc(tsem, 1)

            def emit_MM_half(b, h):
                """One accumulation group (half h of block b)."""
                blk = blocks[b]
                ci = blk["ci"]
                d = CLUSTERS[ci][2]
                nch = max(d // 128, 1)
                kd = min(d, 128)
                mm = blk["mm"]
                if h == 0 and b >= NPS:
                    # psO reuse: casts of block b-NPS must be done
                    te.wait_ge(vcsem, b - NPS + 1)
                    te.wait_ge(acsem, b - NPS + 1)
                seen = plan.setdefault("_pj_seen", set())
                last = None
                for c in range(nch):
                    if h == 0:
                        te.wait_ge(csem, cum_copies[b] + c + 1)
                    i_pj = pj_need(ci, c)
                    if (ci, i_pj) not in seen:
                        te.wait_ge(psem[i_pj], 16)
                        seen.add((ci, i_pj))
                    last = te.matmul(
                        psO[b % NPS][:mm, h * 512:(h + 1) * 512],
                        ET[b % NPS][:kd, c * 128: c * 128 + mm],
                        pjt_sb[ci][:kd, c * D_PROJ + h * 512:
                                   c * D_PROJ + h * 512 + 512],
                        start=(c == 0),
                        stop=(c == nch - 1),
                    )
                last.then_inc(mmsem, 1)

            # software pipeline: T(0) up front; then per block b the two
            # matmul groups with the NEXT block's transposes emitted at the
            # group boundaries (PSUM groups never interleave).
            nb = len(blocks)
            emit_T_chunks(0, 0, cum_copies[1] - cum_copies[0])
            for b in range(nb):
                nch_next = (cum_copies[b + 2] - cum_copies[b + 1]
                            if b + 1 < nb else 0)
                emit_MM_half(b, 0)
                if nch_next:
                    emit_T_chunks(b + 1, 0, (nch_next + 1) // 2)
                emit_MM_half(b, 1)
                if nch_next:
                    emit_T_chunks(b + 1, (nch_next + 1) // 2, nch_next)

        def section_dve(ve, plan):
            blocks = plan["blocks"]
            NPT = 4
            nt = 0

            def copy_chunks(b):
                nonlocal nt
                blk = blocks[b]
                d = CLUSTERS[blk["ci"]][2]
                nch = max(d // 128, 1)
                for c in range(nch):
                    w = min(128, d - c * 128)
                    ve.wait_ge(tsem, nt + 1)
                    ve.tensor_copy(
                        ET[b % NPS][:w, c * 128:(c + 1) * 128],
                        psT[nt % NPT][:w, 0:128],
                    ).then_inc(csem, 1)
                    nt += 1

            # copies first, then the cast of the previous block: keeps the
            # per-block PE<->DVE chain shorter than the gather cadence
            copy_chunks(0)
            for b, blk in enumerate(blocks):
                mm = blk["mm"]
                if b + 1 < len(blocks):
                    copy_chunks(b + 1)
                ve.wait_ge(mmsem, 2 * b + 1)
                ve.tensor_copy(
                    OG[:mm, b * D_PROJ: b * D_PROJ + 512],
                    psO[b % NPS][:mm, 0:512],
                ).then_inc(vcsem, 1)

        def section_act(sc, plan):
            blocks = plan["blocks"]
            for b, blk in enumerate(blocks):
                mm = blk["mm"]
                sc.wait_ge(mmsem, 2 * (b + 1))
                sc.copy(
                    OG[:mm, b * D_PROJ + 512: b * D_PROJ + 1024],
                    psO[b % NPS][:mm, 512:1024],
                ).then_inc(acsem, 1)

        def dispatch(eng, emit):
            if NO_DISPATCH:
                emit(eng, plans[0])
                return
            pid = eng.partition_id()
            with eng.If_lt(pid, 4):
                with eng.If_lt(pid, 2):
                    with eng.If_eq(pid, 0):
                        emit(eng, plans[0])
                    with eng.Else():
                        emit(eng, plans[1])
                with eng.Else():
                    with eng.If_eq(pid, 2):
                        emit(eng, plans[2])
                    with eng.Else():
                        emit(eng, plans[3])
            with eng.Else():
                with eng.If_lt(pid, 6):
                    with eng.If_eq(pid, 4):
                        emit(eng, plans[4])
                    with eng.Else():
                        emit(eng, plans[5])
                with eng.Else():
                    with eng.If_eq(pid, 6):
                        emit(eng, plans[6])
                    with eng.Else():
                        emit(eng, plans[7])

        def _(sy):
            # idx + identity loads are identical on every core: issue them
            # before the dispatch tree so they are not delayed by the
            # partition-id load
            sy.dma_start(idxt[:, :], idxd[:, :]).then_inc(isem, 16)
            sy.dma_start(idn[:, :], iden[:, :]).then_inc(idsem, 16)
            dispatch(sy, section_sync)
        _(nc.sync)

        def _(gp):
            dispatch(gp, section_pool)
        _(nc.gpsimd)

        def _(te):
            def emit(eng, plan):
                plan.pop("_pj_seen", None)
                section_pe(eng, plan)
            dispatch(te, emit)
        _(nc.tensor)

        def _(ve):
            dispatch(ve, section_dve)
        _(nc.vector)

        def _(sc):
            dispatch(sc, section_act)
        _(nc.scalar)

        nc.compile()
    return nc


def kernel(input, emb0, emb1, emb2, emb3, proj0, proj1, proj2, proj3):
    global LAST_RESULT
    inp = np.asarray(input)
    flat = inp.reshape(-1).astype(np.int64)
    T = flat.shape[0]
    tables = [np.asarray(emb0), np.asarray(emb1), np.asarray(emb2),
              np.asarray(emb3)]
    projs = [np.asarray(proj0), np.asarray(proj1), np.asarray(proj2),
             np.asarray(proj3)]

    rt = _route(flat)
    cores = _balance(rt)

    # Cores 3 and 7 showed a consistent extra store-drain latency on HW:
    # hand them the two lightest plans.
    def plan_cost(c):
        return max(POOL_START + c["pool"], PE_START + c["pe"],
                   DMA_START + (c["dma"] + sum(PJ_BYTES[p] for p in c["pj"]))
                   * DMA_NS_PER_BYTE)

    order = sorted(range(N_CORES), key=lambda k: -plan_cost(cores[k]))
    pid_pref = [1, 2, 4, 5, 6, 0, 3, 7]  # heaviest five -> 1,2,4,5,6
    perm = [None] * N_CORES
    for rank, k in enumerate(order):
        perm[pid_pref[rank]] = k
    cores = [cores[perm[pid]] for pid in range(N_CORES)]

    # --- build per-core plans ---------------------------------------------
    plans = []
    for k in range(N_CORES):
        c = cores[k]
        blocks = []
        goff = 0
        tab_lo = {}  # cluster -> (lo_loc, hi_loc)
        for (ci, s, e) in c["blocks"]:
            loc = rt[ci]["loc"][s:e]
            lo, hi = tab_lo.get(ci, (1 << 60, -1))
            tab_lo[ci] = (min(lo, int(loc.min())), max(hi, int(loc.max())))
        plan = {"pj": c["pj"], "blocks": [], "tab_rows": [0] * 4,
                "tab_base": {}}
        for ci, (lo, hi) in tab_lo.items():
            plan["tab_base"][ci] = lo
            plan["tab_rows"][ci] = hi - lo + 1
        for (ci, s, e) in c["blocks"]:
            d = CLUSTERS[ci][2]
            plan["blocks"].append({
                "ci": ci, "s": s, "e": e, "mm": e - s, "goff": goff,
            })
            goff += d
        assert goff <= G_COLS, f"core {k}: G overflow {goff}"
        assert len(plan["blocks"]) <= NB_MAX
        plans.append(plan)

    nc = _build(plans)

    # --- stage host data ---------------------------------------------------
    tab_rows_max = [max(max((p["tab_rows"][ci] for p in plans)), 1)
                    for ci in range(4)]
    pjt_stage = []
    for ci in range(4):
        d = CLUSTERS[ci][2]
        pt = projs[ci].T.astype(np.float32) * EMB_SCALE  # [d, D_PROJ]
        if d >= 128:
            nch = d // 128
            pt = pt.reshape(nch, 128, D_PROJ).transpose(1, 0, 2)
            pt = pt.reshape(128, nch * D_PROJ)
        pjt_stage.append(np.ascontiguousarray(pt.astype(BF16)))
    iden_np = np.eye(128, dtype=np.float32).astype(BF16)

    in_maps = []
    for k in range(N_CORES):
        plan = plans[k]
        mm = {"iden": iden_np}
        for ci in range(4):
            rows = tab_rows_max[ci]
            d = CLUSTERS[ci][2]
            arr = np.zeros((rows, d), dtype=BF16)
            if plan["tab_rows"][ci] > 0:
                base = plan["tab_base"][ci]
                n = plan["tab_rows"][ci]
                arr[:n] = tables[ci][base: base + n].astype(BF16)
            mm[f"tab{ci}"] = arr
            mm[f"pjt{ci}"] = pjt_stage[ci]
        idx = np.zeros((128, NB_MAX), dtype=np.int32)
        for b, blk in enumerate(plan["blocks"]):
            ci = blk["ci"]
            loc = rt[ci]["loc"][blk["s"]: blk["e"]] - plan["tab_base"][ci]
            idx[: blk["mm"], b] = loc.astype(np.int32)
        mm["idxd"] = idx
        in_maps.append(mm)

    res = run_bass_kernel_spmd(nc, in_maps, core_ids=list(range(N_CORES)))
    LAST_RESULT = res

    # --- unpermute ---------------------------------------------------------
    out_full = np.zeros((T, D_PROJ), np.float32)
    for k in range(N_CORES):
        rows = res.results[k]["out"]
        for b, blk in enumerate(plans[k]["blocks"]):
            ci = blk["ci"]
            pos = rt[ci]["pos"][blk["s"]: blk["e"]]
            out_full[pos] = rows[b * 128: b * 128 + blk["mm"]].astype(
                np.float32)
    return out_full.reshape(*inp.shape, D_PROJ)


# revision 37
# speedup vs baseline: 1.1387x; 1.0066x over previous
"""Adaptive embedding (nn_AdaptiveEmbedding) Trainium2 Bass kernel, v2.

Design: one SPMD program with per-core specialized sections dispatched via a
partition_id() If-tree.  Host routes tokens to cores so each core serves a
small set of clusters (cluster-specialized sharding) -- this removes the 8x
replication of the projection matrices that dominated HBM traffic in v1.

Per 128-token block, the device:
  Pool : one indirect_dma_start (HW dynamic-offset DMA, no ucode library)
         gathering 128 table rows -> SBUF [tokens, d] (token per partition)
  PE   : transposes the gathered tile chunk-wise to [d, tokens] (via identity
         matmul into bf16 PSUM), then runs the projection matmuls into f32
         PSUM, software-pipelined one block ahead of the transposes
  DVE  : copies transposed chunks PSUM->SBUF and casts output half 0
  ACT  : casts output half 1
  SP   : loads idx/identity/projection tiles, stores finished blocks

Host scatters per-core block outputs back to original token positions.
"""

from contextlib import ExitStack

import numpy as np
import ml_dtypes

import concourse.bacc as bacc
import concourse.bass as bass
import concourse.mybir as mybir
from concourse.bass_utils import run_bass_kernel_spmd

N_CORES = 8
D_PROJ = 1024
EMB_SCALE = float(D_PROJ) ** 0.5
BF16 = ml_dtypes.bfloat16

# clusters: (token_left, token_right, d)
CLUSTERS = [
    (0, 20000, 1024),
    (20000, 60000, 256),
    (60000, 100000, 64),
    (100000, 128000, 16),
]

# cost model for the balancer (ns)
POOL_PER_BLOCK = 1410.0
PE_NS_PER_COL = 0.55         # p-state mix
DMA_NS_PER_BYTE = 1.0 / 300.0e9 * 1e9   # 300 GB/s
POOL_START, PE_START, DMA_START = 10500.0, 14500.0, 8300.0
VEC_START = 14500.0
# max number of cores each cluster's blocks (and proj copy) may spread to
SPREAD_CAP = [4, 3, 4, 6]


def _dve_ns(d):
    return max(d // 128, 1) * 300.0 + 530.0  # T-copies + h0 cast

NB_MAX = 16          # max blocks per core the program supports
G_COLS = 8192        # gather buffer cols (bf16) per partition
NPS = 2              # psum double-buffer depth (out tiles and T tiles)
NO_DISPATCH = False  # debug: emit plans[0] for every core, no branching

LAST_RESULT = None


def _pe_cols(d):
    nch = max(d // 128, 1)
    kd = min(d, 128)
    return nch * 128 + nch * 2 * 512  # transposes + matmuls (free-dim cols)


def _block_bytes(d):
    return 128 * d * 2 + 128 * D_PROJ * 2  # gather + out (bf16)


PJ_BYTES = [2 * 1024 * 1024, 512 * 1024, 128 * 1024, 32 * 1024]


def _route(flat):
    """Token routing per cluster: sorted positions and local indices."""
    out = []
    for (l, r, d) in CLUSTERS:
        sel = (flat >= l) & (flat < r)
        pos = np.nonzero(sel)[0]
        loc = (flat[pos] - l).astype(np.int64)
        order = np.argsort(loc, kind="stable")
        out.append({"pos": pos[order], "loc": loc[order], "d": d, "n": len(pos)})
    return out


def _balance(rt):
    """Block assignment minimizing max per-core makespan, with a hard cap on
    how many cores each cluster (and its proj copy) may spread to.

    Returns cores: list of 8 dicts with
      blocks: list of (cluster, start, end) token ranges (<=128 each)
      pj: set of cluster ids present
    Token ranges index into the cluster's sorted token arrays, so each
    core gets a contiguous slice of the sorted-by-loc token list (compact
    vocab slice per core).
    """
    cores = [{"blocks": [], "pj": set(), "pool": 0.0, "pe": 0.0, "dma": 0.0,
              "dve": 0.0, "nb": 0}
             for _ in range(N_CORES)]

    def span(c, dpool=0.0, dpe=0.0, ddma=0.0, pj_extra=0, ddve=0.0):
        pjb = sum(PJ_BYTES[p] for p in c["pj"]) + pj_extra
        return max(POOL_START + c["pool"] + dpool,
                   PE_START + c["pe"] + dpe,
                   VEC_START + c["dve"] + ddve,
                   DMA_START + (c["dma"] + ddma + pjb) * DMA_NS_PER_BYTE)

    for ci in [0, 1, 2, 3]:
        n = rt[ci]["n"]
        nblk = (n + 127) // 128
        d = rt[ci]["d"]
        pe_b = _pe_cols(d) * PE_NS_PER_COL
        by_b = _block_bytes(d)
        dve_b = _dve_ns(d)
        # spread floor so per-core G columns and block counts stay in range
        blocks_cap = max(min(G_COLS // d, NB_MAX) - 2, 1)
        spread = max(SPREAD_CAP[ci], -(-nblk // blocks_cap))
        counts = [0] * N_CORES
        for _ in range(nblk):
            # candidate cores: those already serving ci, or (if spread cap
            # not hit) any core.  Cores serving c0 are PE/DMA-heavy: cap
            # their total block count so the gather cadence of small blocks
            # does not stack on top of the c0 matmul load.
            have = [k for k in range(N_CORES)
                    if ci in cores[k]["pj"]
                    and counts[k] < blocks_cap
                    and cores[k]["nb"] < NB_MAX - 1]
            cands = (list(range(N_CORES)) if len(have) < spread else have)
            cands = [k for k in cands
                     if counts[k] < blocks_cap
                     and cores[k]["nb"] < NB_MAX - 1] or \
                    [k for k in range(N_CORES) if cores[k]["nb"] < NB_MAX - 1]
            if ci != 0:
                cands = [k for k in cands
                         if not (0 in cores[k]["pj"]
                                 and cores[k]["nb"] >= 6)] or cands
            best, bestv = None, None
            for k in cands:
                c = cores[k]
                extra_pj = 0 if ci in c["pj"] else PJ_BYTES[ci]
                v = span(c, POOL_PER_BLOCK, pe_b, by_b, extra_pj, dve_b)
                # tie-break: prefer cores that already carry this proj
                v += (0 if ci in c["pj"] else 1.0)
                if bestv is None or v < bestv - 1e-9:
                    bestv, best = v, k
            c = cores[best]
            c["pool"] += POOL_PER_BLOCK
            c["pe"] += pe_b
            c["dma"] += by_b
            c["dve"] += dve_b
            c["pj"].add(ci)
            c["nb"] += 1
            counts[best] += 1
        start = 0
        for k in range(N_CORES):
            for _ in range(counts[k]):
                end = min(start + 128, n)
                cores[k]["blocks"].append((ci, start, end))
                start = end
        assert start == n
    return cores


def _build(plans):
    """plans[k]: list of block descriptors:
       (cluster, d, tab_row_offset_base, nblk_index, mm) plus idx data handled
       by host.  We need per-core: blocks list with (cluster, mm)."""
    nc = bacc.Bacc("TRN2", target_bir_lowering=False, num_devices=N_CORES)

    # table shapes: max rows over cores per cluster (host pads)
    tab_rows = [max((p["tab_rows"][ci] for p in plans), default=1) or 1
                for ci in range(4)]
    tabs = [nc.dram_tensor(f"tab{ci}", [max(tab_rows[ci], 1), CLUSTERS[ci][2]],
                           mybir.dt.bfloat16, kind="ExternalInput")
            for ci in range(4)]
    idxd = nc.dram_tensor("idxd", [128, NB_MAX], mybir.dt.int32,
                          kind="ExternalInput")
    iden = nc.dram_tensor("iden", [128, 128], mybir.dt.bfloat16,
                          kind="ExternalInput")
    pjts = [nc.dram_tensor(f"pjt{ci}", [min(CLUSTERS[ci][2], 128),
                                        max(CLUSTERS[ci][2] // 128, 1) * D_PROJ],
                           mybir.dt.bfloat16, kind="ExternalInput")
            for ci in range(4)]
    outD = nc.dram_tensor("out", [NB_MAX * 128, D_PROJ], mybir.dt.bfloat16,
                          kind="ExternalOutput")

    stack = ExitStack()
    sb = lambda name, shape, dt: stack.enter_context(
        nc.sbuf_tensor(name, list(shape), dt))
    pt_ = lambda name, shape, dt: stack.enter_context(
        nc.psum_tensor(name, list(shape), dt))
    sem = lambda name: stack.enter_context(nc.semaphore(name))

    with stack:
        idxt = sb("idxt", [128, NB_MAX], mybir.dt.int32)
        idn = sb("idn", [128, 128], mybir.dt.bfloat16)
        G = sb("G", [128, G_COLS], mybir.dt.bfloat16)
        ET = [sb(f"ET{i}", [128, 1024], mybir.dt.bfloat16) for i in range(NPS)]
        OG = sb("OG", [128, NB_MAX * D_PROJ], mybir.dt.bfloat16)
        pjt_sb = [sb(f"pj{ci}", [min(CLUSTERS[ci][2], 128),
                                 max(CLUSTERS[ci][2] // 128, 1) * D_PROJ],
                     mybir.dt.bfloat16) for ci in range(4)]
        # transposes must land at a PSUM bank base: rotate four bank-sized
        # slots, each transpose writes cols 0:128 of its slot
        NPT = 4
        psT = [pt_(f"psT{i}", [128, 1024], mybir.dt.bfloat16)
               for i in range(NPT)]
        psO = [pt_(f"psO{i}", [128, D_PROJ], mybir.dt.float32)
               for i in range(NPS)]

        isem = sem("isem")    # idx load
        idsem = sem("idsem")  # identity load
        psem = [sem(f"psem{i}") for i in range(7)]   # proj tile loads
        gsem = [sem(f"gsem{i}") for i in range(NB_MAX)]  # per-block gathers
        tsem = sem("tsem")    # PE transposes
        csem = sem("csem")    # DVE chunk copies
        mmsem = sem("mmsem")  # matmul halves
        vcsem = sem("vcsem")  # DVE out casts (h0)
        acsem = sem("acsem")  # ACT out casts (h1)
        osem = sem("osem")    # stores

        # per-core proj DMA schedule: list of (cluster, chunk_lo, n_chunks)
        # c0 is split into 4 DMAs of 2 chunks; others one DMA each.
        def proj_dmas(pjset):
            sched = []
            for ci in sorted(pjset):
                nch = max(CLUSTERS[ci][2] // 128, 1)
                if ci == 0:
                    for c0 in range(0, nch, 2):
                        sched.append((ci, c0, 2))
                else:
                    sched.append((ci, 0, nch))
            return sched

        def section_sync(sy, plan):
            for i, (ci, c0, w) in enumerate(proj_dmas(plan["pj"])):
                part = min(CLUSTERS[ci][2], 128)
                sy.dma_start(
                    pjt_sb[ci][:part, c0 * D_PROJ:(c0 + w) * D_PROJ],
                    pjts[ci][:part, c0 * D_PROJ:(c0 + w) * D_PROJ],
                ).then_inc(psem[i], 16)
            # stores: full-width for all but the last block; the last block
            # ships each half as soon as its cast lands (shorter tail)
            nb = len(plan["blocks"])
            nst = 0
            for b, blk in enumerate(plan["blocks"]):
                if b < nb - 1:
                    sy.wait_ge(vcsem, b + 1)
                    sy.wait_ge(acsem, b + 1)
                    sy.dma_start(
                        outD[b * 128: b * 128 + blk["mm"], :],
                        OG[:blk["mm"], b * D_PROJ:(b + 1) * D_PROJ],
                    ).then_inc(osem, 16)
                    nst += 1
                else:
                    sy.wait_ge(vcsem, b + 1)
                    sy.dma_start(
                        outD[b * 128: b * 128 + blk["mm"], 0:512],
                        OG[:blk["mm"], b * D_PROJ: b * D_PROJ + 512],
                    ).then_inc(osem, 16)
                    sy.wait_ge(acsem, b + 1)
                    sy.dma_start(
                        outD[b * 128: b * 128 + blk["mm"], 512:1024],
                        OG[:blk["mm"], b * D_PROJ + 512:(b + 1) * D_PROJ],
                    ).then_inc(osem, 16)
                    nst += 2
            sy.wait_ge(osem, 16 * nst)

        def section_pool(gp, plan):
            gp.wait_ge(isem, 16)
            for b, blk in enumerate(plan["blocks"]):
                ci = blk["ci"]
                d = CLUSTERS[ci][2]
                gp.indirect_dma_start(
                    G[:, blk["goff"]: blk["goff"] + d], None,
                    tabs[ci][:, :],
                    bass.IndirectOffsetOnAxis(ap=idxt[:, b:b + 1], axis=0),
                ).then_inc(gsem[b], 16)

        def section_pe(te, plan):
            blocks = plan["blocks"]
            pj_sched = proj_dmas(plan["pj"])
            # dma index (0-based) needed for cluster ci chunk c
            def pj_need(ci, c):
                for i, (cj, c0, w) in enumerate(pj_sched):
                    if cj == ci and c0 <= c < c0 + w:
                        return i
                raise AssertionError

            te.wait_ge(idsem, 16)

            cum_copies = [0] * (len(blocks) + 1)
            for b, blk in enumerate(blocks):
                d = CLUSTERS[blk["ci"]][2]
                cum_copies[b + 1] = cum_copies[b] + max(d // 128, 1)

            def emit_T_chunks(b, c_lo, c_hi):
                blk = blocks[b]
                d = CLUSTERS[blk["ci"]][2]
                if c_lo == 0:
                    te.wait_ge(gsem[b], 16)
                for c in range(c_lo, c_hi):
                    w = min(128, d - c * 128)
                    t = cum_copies[b] + c
                    if t >= NPT:
                        te.wait_ge(csem, t - (NPT - 1))
                    te.transpose(
                        psT[t % NPT][:w, 0:128],
                        G[:, blk["goff"] + c * 128: blk["goff"] + c * 128 + w],
                        idn[:, :],
                    ).then_inc(tsem, 1)

            def emit_MM_half(b, h):
                """One accumulation group (half h of block b)."""
                blk = blocks[b]
                ci = blk["ci"]
                d = CLUSTERS[ci][2]
                nch = max(d // 128, 1)
                kd = min(d, 128)
                mm = blk["mm"]
                if h == 0 and b >= NPS:
                    # psO reuse: casts of block b-NPS must be done
                    te.wait_ge(vcsem, b - NPS + 1)
                    te.wait_ge(acsem, b - NPS + 1)
                seen = plan.setdefault("_pj_seen", set())
                last = None
                for c in range(nch):
                    if h == 0:
                        te.wait_ge(csem, cum_copies[b] + c + 1)
                    i_pj = pj_need(ci, c)
                    if (ci, i_pj) not in seen:
                        te.wait_ge(psem[i_pj], 16)
                        seen.add((ci, i_pj))
                    last = te.matmul(
                        psO[b % NPS][:mm, h * 512:(h + 1) * 512],
                        ET[b % NPS][:kd, c * 128: c * 128 + mm],
                        pjt_sb[ci][:kd, c * D_PROJ + h * 512:
                                   c * D_PROJ + h * 512 + 512],
                        start=(c == 0),
                        stop=(c == nch - 1),
                    )
                last.then_inc(mmsem, 1)

            # software pipeline: T(0) up front; then per block b the two
            # matmul groups with the NEXT block's transposes emitted at the
            # group boundaries (PSUM groups never interleave).
            nb = len(blocks)
            emit_T_chunks(0, 0, cum_copies[1] - cum_copies[0])
            for b in range(nb):
                nch_next = (cum_copies[b + 2] - cum_copies[b + 1]
                            if b + 1 < nb else 0)
                emit_MM_half(b, 0)
                if nch_next:
                    emit_T_chunks(b + 1, 0, (nch_next + 1) // 2)
                emit_MM_half(b, 1)
                if nch_next:
                    emit_T_chunks(b + 1, (nch_next + 1) // 2, nch_next)

        def section_dve(ve, plan):
            blocks = plan["blocks"]
            NPT = 4
            nt = 0

            def copy_chunks(b):
                nonlocal nt
                blk = blocks[b]
                d = CLUSTERS[blk["ci"]][2]
                nch = max(d // 128, 1)
                for c in range(nch):
                    w = min(128, d - c * 128)
                    ve.wait_ge(tsem, nt + 1)
                    ve.tensor_copy(
                        ET[b % NPS][:w, c * 128:(c + 1) * 128],
                        psT[nt % NPT][:w, 0:128],
                    ).then_inc(csem, 1)
                    nt += 1

            # copies first, then the cast of the previous block: keeps the
            # per-block PE<->DVE chain shorter than the gather cadence
            copy_chunks(0)
            for b, blk in enumerate(blocks):
                mm = blk["mm"]
                if b + 1 < len(blocks):
                    copy_chunks(b + 1)
                ve.wait_ge(mmsem, 2 * b + 1)
                ve.tensor_copy(
                    OG[:mm, b * D_PROJ: b * D_PROJ + 512],
                    psO[b % NPS][:mm, 0:512],
                ).then_inc(vcsem, 1)

        def section_act(sc, plan):
            blocks = plan["blocks"]
            for b, blk in enumerate(blocks):
                mm = blk["mm"]
                sc.wait_ge(mmsem, 2 * (b + 1))
                sc.copy(
                    OG[:mm, b * D_PROJ + 512: b * D_PROJ + 1024],
                    psO[b % NPS][:mm, 512:1024],
                ).then_inc(acsem, 1)

        def dispatch(eng, emit):
            if NO_DISPATCH:
                emit(eng, plans[0])
                return
            pid = eng.partition_id()
            with eng.If_lt(pid, 4):
                with eng.If_lt(pid, 2):
                    with eng.If_eq(pid, 0):
                        emit(eng, plans[0])
                    with eng.Else():
                        emit(eng, plans[1])
                with eng.Else():
                    with eng.If_eq(pid, 2):
                        emit(eng, plans[2])
                    with eng.Else():
                        emit(eng, plans[3])
            with eng.Else():
                with eng.If_lt(pid, 6):
                    with eng.If_eq(pid, 4):
                        emit(eng, plans[4])
                    with eng.Else():
                        emit(eng, plans[5])
                with eng.Else():
                    with eng.If_eq(pid, 6):
                        emit(eng, plans[6])
                    with eng.Else():
                        emit(eng, plans[7])

        def _(sy):
            # idx + identity loads are identical on every core: issue them
            # before the dispatch tree so they are not delayed by the
            # partition-id load
            sy.dma_start(idxt[:, :], idxd[:, :]).then_inc(isem, 16)
            sy.dma_start(idn[:, :], iden[:, :]).then_inc(idsem, 16)
            dispatch(sy, section_sync)
        _(nc.sync)

        def _(gp):
            dispatch(gp, section_pool)
        _(nc.gpsimd)

        def _(te):
            def emit(eng, plan):
                plan.pop("_pj_seen", None)
                section_pe(eng, plan)
            dispatch(te, emit)
        _(nc.tensor)

        def _(ve):
            dispatch(ve, section_dve)
        _(nc.vector)

        def _(sc):
            dispatch(sc, section_act)
        _(nc.scalar)

        nc.compile()
    return nc


def kernel(input, emb0, emb1, emb2, emb3, proj0, proj1, proj2, proj3):
    global LAST_RESULT
    inp = np.asarray(input)
    flat = inp.reshape(-1).astype(np.int64)
    T = flat.shape[0]
    tables = [np.asarray(emb0), np.asarray(emb1), np.asarray(emb2),
              np.asarray(emb3)]
    projs = [np.asarray(proj0), np.asarray(proj1), np.asarray(proj2),
             np.asarray(proj3)]

    rt = _route(flat)
    cores = _balance(rt)

    # Cores 3 and 7 showed a consistent extra store-drain latency on HW:
    # hand them the two lightest plans.
    def plan_cost(c):
        return max(POOL_START + c["pool"], PE_START + c["pe"],
                   DMA_START + (c["dma"] + sum(PJ_BYTES[p] for p in c["pj"]))
                   * DMA_NS_PER_BYTE)

    order = sorted(range(N_CORES), key=lambda k: -plan_cost(cores[k]))
    pid_pref = [1, 2, 4, 5, 6, 0, 3, 7]  # heaviest five -> 1,2,4,5,6
    perm = [None] * N_CORES
    for rank, k in enumerate(order):
        perm[pid_pref[rank]] = k
    cores = [cores[perm[pid]] for pid in range(N_CORES)]

    # --- build per-core plans ---------------------------------------------
    plans = []
    for k in range(N_CORES):
        c = cores[k]
        blocks = []
        goff = 0
        tab_lo = {}  # cluster -> (lo_loc, hi_loc)
        for (ci, s, e) in c["blocks"]:
            loc = rt[ci]["loc"][s:e]
            lo, hi = tab_lo.get(ci, (1 << 60, -1))
            tab_lo[ci] = (min(lo, int(loc.min())), max(hi, int(loc.max())))
        plan = {"pj": c["pj"], "blocks": [], "tab_rows": [0] * 4,
                "tab_base": {}}
        for ci, (lo, hi) in tab_lo.items():
            plan["tab_base"][ci] = lo
            plan["tab_rows"][ci] = hi - lo + 1
        for (ci, s, e) in c["blocks"]:
            d = CLUSTERS[ci][2]
            plan["blocks"].append({
                "ci": ci, "s": s, "e": e, "mm": e - s, "goff": goff,
            })
            goff += d
        assert goff <= G_COLS, f"core {k}: G overflow {goff}"
        assert len(plan["blocks"]) <= NB_MAX
        plans.append(plan)

    nc = _build(plans)

    # --- stage host data ---------------------------------------------------
    tab_rows_max = [max(max((p["tab_rows"][ci] for p in plans)), 1)
                    for ci in range(4)]
    pjt_stage = []
    for ci in range(4):
        d = CLUSTERS[ci][2]
        pt = projs[ci].T.astype(np.float32) * EMB_SCALE  # [d, D_PROJ]
        if d >= 128:
            nch = d // 128
            pt = pt.reshape(nch, 128, D_PROJ).transpose(1, 0, 2)
            pt = pt.reshape(128, nch * D_PROJ)
        pjt_stage.append(np.ascontiguousarray(pt.astype(BF16)))
    iden_np = np.eye(128, dtype=np.float32).astype(BF16)

    in_maps = []
    for k in range(N_CORES):
        plan = plans[k]
        mm = {"iden": iden_np}
        for ci in range(4):
            rows = tab_rows_max[ci]
            d = CLUSTERS[ci][2]
            arr = np.zeros((rows, d), dtype=BF16)
            if plan["tab_rows"][ci] > 0:
                base = plan["tab_base"][ci]
                n = plan["tab_rows"][ci]
                arr[:n] = tables[ci][base: base + n].astype(BF16)
            mm[f"tab{ci}"] = arr
            mm[f"pjt{ci}"] = pjt_stage[ci]
        idx = np.zeros((128, NB_MAX), dtype=np.int32)
        for b, blk in enumerate(plan["blocks"]):
            ci = blk["ci"]
            loc = rt[ci]["loc"][blk["s"]: blk["e"]] - plan["tab_base"][ci]
            idx[: blk["mm"], b] = loc.astype(np.int32)
        mm["idxd"] = idx
        in_maps.append(mm)

    res = run_bass_kernel_spmd(nc, in_maps, core_ids=list(range(N_CORES)))
    LAST_RESULT = res

    # --- unpermute ---------------------------------------------------------
    out_full = np.zeros((T, D_PROJ), np.float32)
    for k in range(N_CORES):
        rows = res.results[k]["out"]
        for b, blk in enumerate(plans[k]["blocks"]):
            ci = blk["ci"]
            pos = rt[ci]["pos"][blk["s"]: blk["e"]]
            out_full[pos] = rows[b * 128: b * 128 + blk["mm"]].astype(
                np.float32)
    return out_full.reshape(*inp.shape, D_PROJ)
